# revision 10
# baseline (speedup 1.0000x reference)
"""AdaptiveRankChristoffel kernel for one TRN2 chip (8 NeuronCores).

Data-parallel over tokens: v [4,8192,512] -> 32768 tokens, 4096 per core.
Host pre-transposes v to dim-major fp16 [512, tokens] so the device streams
it straight into matmuls (contraction dim on partitions, no on-chip
transpose); output is written fp16 and widened on the host. All heavy
matmuls run in fp16/f32r at full PE rate; everything accumulates in fp32.

Device pipeline per core:
  pass 1  : stream vT slabs; fused [U|w1] fp16 matmul gives proj + h in one
            PSUM tile; ACT Square writes proj^2 (squn, f32r) straight to
            SBUF; relu -> w2 matmul -> tanh(z/2) (sigmoid rewritten via
            tanh; same ACT table as relu) with accum_out partial sums.
            Interleaved: norm2 candidates for ALL 61 possible eff_ranks
            (static prefix-mask rhs, f32r) -- mask-independent, so it hides
            under pass 1 + the collective.
  exchange: per-core sum -> AllToAll gather (cheaper than AllReduce here)
            -> PE ones-matmul sum -> e = 35.2 + S*c.
  post    : rank mask via exact integer-threshold compares; norm2 column
            selected from the 61 candidates with a register-backed dynamic
            slice; scale = 0.1/(1+sqrt(norm2)+eps).
  pass 2  : m2 = squn*mask (f32r); gamma [128,512] = m2^T @ W^T per
            128-token chunk; out = 10*tanh(gamma*scale) via per-partition
            AP scale on ACT; fp16 out stream.
"""

import sys

sys.path.insert(0, "/opt/trn_rl_repo")

import numpy as np

BATCH, SEQ, DIM = 4, 8192, 512
MAX_RANK = 64
HID = 32
NCORES = 8
TOKENS = BATCH * SEQ            # 32768
T = TOKENS // NCORES            # 4096 tokens per core
SLAB = 512                      # tokens per slab
NSLAB = T // SLAB               # 8
CHUNK = 128                     # tokens per gamma matmul
NCHUNK = T // CHUNK             # 32
KC = DIM // 128                 # 4 contraction chunks
NCAND = 64                      # norm2 candidate columns (eff = 4+min(c,60))

EPS = 1e-8
CLAMP = 10.0
# e = 64*avg_ratio = 35.2 + S * (57.6/65536), S = global sum of tanh(z/2)
E_SCALE = 57.6 / 65536.0
E_BIAS = 35.2

_nc_cache = None
_last_in_maps = None


def _build():
    from concourse import bacc, bass, mybir, tile

    f32 = mybir.dt.float32
    f32r = mybir.dt.float32r
    fp16 = mybir.dt.float16
    i32 = mybir.dt.int32
    AF = mybir.ActivationFunctionType
    ALU = mybir.AluOpType

    nc = bacc.Bacc(None, debug=False)

    vt = nc.declare_dram_parameter("vt", [DIM, T], fp16, isOutput=False)
    uw1 = nc.declare_dram_parameter("uw1", [DIM, MAX_RANK + HID], fp16, isOutput=False)
    wt = nc.declare_dram_parameter("wt", [MAX_RANK, DIM], f32, isOutput=False)
    w2 = nc.declare_dram_parameter("w2", [HID, 1], fp16, isOutput=False)
    b1 = nc.declare_dram_parameter("b1", [HID, 1], f32, isOutput=False)
    b2h = nc.declare_dram_parameter("b2h", [1, 1], f32, isOutput=False)
    iota = nc.declare_dram_parameter("iota", [MAX_RANK, 1], f32, isOutput=False)
    onesrow = nc.declare_dram_parameter("onesrow", [1, MAX_RANK], f32, isOutput=False)
    ones8 = nc.declare_dram_parameter("ones8", [NCORES, 2], f32, isOutput=False)
    pfx = nc.declare_dram_parameter("pfx", [MAX_RANK, NCAND], f32, isOutput=False)
    out = nc.declare_dram_parameter("out", [T, DIM], fp16, isOutput=True)

    with tile.TileContext(nc) as tc:
        with (
            tc.tile_pool(name="persist", bufs=1) as pp,
            tc.tile_pool(name="vtp", bufs=4) as vtp,
            tc.tile_pool(name="small", bufs=2) as sp,
            tc.tile_pool(name="outp", bufs=2) as op_,
            tc.tile_pool(name="ps1", bufs=3, space="PSUM") as ps1p,
            tc.tile_pool(name="ps2", bufs=2, space="PSUM") as ps2p,
            tc.tile_pool(name="psg", bufs=3, space="PSUM") as psgp,
            tc.tile_pool(name="dram", bufs=1, space="DRAM") as dram,
        ):
            # ---- constants ----
            uw1t = pp.tile([128, KC, MAX_RANK + HID], fp16, tag="uw1t")
            nc.sync.dma_start(uw1t[:], uw1[:].rearrange("(c p) m -> p c m", p=128))
            wtr = pp.tile([MAX_RANK, DIM], f32r, tag="wtr")
            nc.gpsimd.dma_start(wtr[:], wt[:])
            w2t = pp.tile([HID, 1], fp16, tag="w2t")
            nc.sync.dma_start(w2t[:], w2[:])
            b1t = pp.tile([HID, 1], f32, tag="b1t")
            nc.sync.dma_start(b1t[:], b1[:])
            b2t = pp.tile([1, 1], f32, tag="b2t")
            nc.sync.dma_start(b2t[:], b2h[:])
            iot = pp.tile([MAX_RANK, 1], f32, tag="iot")
            nc.sync.dma_start(iot[:], iota[:])
            onr = pp.tile([1, MAX_RANK], f32, tag="onr")
            nc.sync.dma_start(onr[:], onesrow[:])
            on8 = pp.tile([NCORES, 2], f32, tag="on8")
            nc.sync.dma_start(on8[:], ones8[:])
            pfxt = pp.tile([MAX_RANK, NCAND], f32r, tag="pfxt")
            nc.gpsimd.dma_start(pfxt[:], pfx[:])

            # ---- persistent state ----
            squn = pp.tile([MAX_RANK, T], f32r, tag="squn")
            m2 = pp.tile([MAX_RANK, T], f32r, tag="m2")
            n2all = pp.tile([128, NCHUNK * NCAND], f32, tag="n2all")
            partials = pp.tile([1, NSLAB], f32, tag="partials")

            # ---- pass 1 (+ interleaved norm2 candidates) ----
            for s in range(NSLAB):
                t0 = s * SLAB
                vslab = vtp.tile([128, KC, SLAB], fp16, tag="vslab")
                src = vt[:, t0 : t0 + SLAB].rearrange("(c p) t -> p c t", p=128)
                nc.sync.dma_start(vslab[:], src)

                ps1 = ps1p.tile([MAX_RANK + HID, SLAB], f32, tag="ps1")
                for c in range(KC):
                    nc.tensor.matmul(
                        ps1[:], lhsT=uw1t[:, c, :], rhs=vslab[:, c, :],
                        start=(c == 0), stop=(c == KC - 1),
                    )
                # squn = proj^2 straight from PSUM (f32r for downstream matmuls)
                nc.scalar.activation(
                    squn[:, t0 : t0 + SLAB], ps1[0:MAX_RANK, :], AF.Square,
                    bias=0.0, scale=1.0,
                )
                # complexity net
                hrel = sp.tile([HID, SLAB], fp16, tag="hrel")
                nc.vector.tensor_scalar(
                    hrel[:], ps1[MAX_RANK : MAX_RANK + HID, :],
                    b1t[:], 0.0, ALU.add, ALU.max,
                )
                ps2 = ps2p.tile([1, SLAB], f32, tag="ps2share")
                nc.tensor.matmul(ps2[:], lhsT=w2t[:], rhs=hrel[:], start=True, stop=True)
                tval = sp.tile([1, SLAB], f32, tag="tval")
                nc.scalar.activation(
                    tval[:], ps2[:], AF.Tanh, bias=b2t[:], scale=0.5,
                    accum_out=partials[0:1, s : s + 1],
                )
            # ---- local sum -> AllToAll gather -> global sum ----
            gloc0 = pp.tile([1, 1], f32, tag="gloc0")
            nc.vector.reduce_sum(gloc0[:], partials[:], axis=mybir.AxisListType.X)
            glp = ps2p.tile([NCORES, 1], f32, tag="ps2share")
            nc.tensor.matmul(glp[:], lhsT=onr[0:1, 0:NCORES], rhs=gloc0[:],
                             start=True, stop=True)
            gloc = pp.tile([NCORES, 1], f32, tag="gloc")
            nc.vector.tensor_copy(gloc[:], glp[:])
            cci = dram.tile([NCORES, 1], f32)
            cco = dram.tile([NCORES, 1], f32)
            nc.gpsimd.dma_start(cci[:], gloc[:])
            nc.gpsimd.collective_compute(
                "AllToAll", ALU.bypass,
                replica_groups=[list(range(NCORES))],
                ins=[cci[:].opt()], outs=[cco[:].opt()],
            )
            gat = pp.tile([NCORES, 1], f32, tag="gat")
            nc.gpsimd.dma_start(gat[:], cco[:])
            gsp = ps2p.tile([1, 2], f32, tag="ps2share")
            nc.tensor.matmul(gsp[:], lhsT=gat[:], rhs=on8[:], start=True, stop=True)
            gsum = pp.tile([1, 1], f32, tag="gsum")
            nc.vector.tensor_copy(gsum[:], gsp[0:1, 0:1])

            # warm the Sqrt ACT table while the collective is in flight
            scratch = pp.tile([1, 1], f32, tag="scratch")
            nc.scalar.activation(scratch[:], gloc0[:], AF.Sqrt, bias=0.0, scale=0.0)

            # norm2 candidates for every chunk x all 61 eff_ranks -- mask-
            # independent, so this PE work hides under the collective stall
            for j in range(NCHUNK):
                n2p = ps1p.tile([128, NCAND], f32, tag="ps1")
                nc.tensor.matmul(
                    n2p[:], lhsT=squn[:, j * CHUNK : (j + 1) * CHUNK],
                    rhs=pfxt[:], start=True, stop=True,
                )
                nc.vector.tensor_copy(
                    n2all[:, j * NCAND : (j + 1) * NCAND], n2p[:]
                )

            # ---- e scalar -> rank mask + norm2 column select ----
            e_t = pp.tile([1, 1], f32, tag="e_t")
            nc.vector.tensor_scalar(e_t[:], gsum[:], E_SCALE, E_BIAS, ALU.mult, ALU.add)
            # mask[r] = (e - iota >= 1) | (iota <= 3)
            ebp = ps2p.tile([MAX_RANK, 1], f32, tag="ps2share")
            nc.tensor.matmul(ebp[:], lhsT=onr[:], rhs=e_t[:], start=True, stop=True)
            eb = pp.tile([MAX_RANK, 1], f32, tag="eb")
            nc.vector.tensor_copy(eb[:], ebp[:])
            d_t = pp.tile([MAX_RANK, 1], f32, tag="d_t")
            nc.vector.tensor_sub(d_t[:], eb[:], iot[:])
            ma = pp.tile([MAX_RANK, 1], f32, tag="ma")
            nc.vector.tensor_scalar(ma[:], d_t[:], 1.0, None, ALU.is_ge)
            mb = pp.tile([MAX_RANK, 1], f32, tag="mb")
            nc.vector.tensor_scalar(mb[:], iot[:], 3.0, None, ALU.is_le)
            mask = pp.tile([MAX_RANK, 1], f32, tag="mask")
            nc.vector.tensor_tensor(mask[:], ma[:], mb[:], ALU.max)

            # idx = clip(int(e) - 4, 0, 60) -> register -> dynamic column pick
            idxf = pp.tile([1, 1], f32, tag="idxf")
            nc.vector.tensor_scalar(idxf[:], e_t[:], -4.0, 0.0, ALU.add, ALU.max)
            idxf2 = pp.tile([1, 1], f32, tag="idxf2")
            nc.vector.tensor_scalar(idxf2[:], idxf[:], 60.0, None, ALU.min)
            idxi = pp.tile([1, 1], i32, tag="idxi")
            nc.vector.tensor_copy(idxi[:], idxf2[:])
            regs = nc.alloc_registers()
            nc.regs_load(regs, idxi[0:1, 0:1])
            sv = nc.snap(regs, donate=True, min_val=0, max_val=NCAND - 1)
            n2 = pp.tile([128, NCHUNK], f32, tag="n2")
            n2view = n2all[:].rearrange("p (j k) -> p j k", k=NCAND)
            nc.vector.tensor_copy(n2[:], n2view[:, :, bass.ds(sv, 1)])

            # scale = 0.1 / (1 + sqrt(n2) + eps), per token chunk column
            nrm = pp.tile([128, NCHUNK], f32, tag="nrm")
            nc.scalar.activation(nrm[:], n2[:], AF.Sqrt, bias=0.0, scale=1.0)
            np1 = pp.tile([128, NCHUNK], f32, tag="np1")
            nc.vector.tensor_scalar(np1[:], nrm[:], 1.0 + EPS, None, ALU.add)
            rcp = pp.tile([128, NCHUNK], f32, tag="rcp")
            nc.vector.reciprocal(rcp[:], np1[:])
            s01 = pp.tile([128, NCHUNK], f32, tag="s01")
            nc.vector.tensor_scalar(s01[:], rcp[:], 1.0 / CLAMP, None, ALU.mult)

            # m2 = squn * mask  (per slab)
            for s in range(NSLAB):
                t0 = s * SLAB
                nc.vector.tensor_scalar(
                    m2[:, t0 : t0 + SLAB], squn[:, t0 : t0 + SLAB],
                    mask[:], None, ALU.mult,
                )

            # ---- pass 2: gamma + tanh + scale-out (fp16) ----
            for s in range(NSLAB):
                ot = op_.tile([128, SLAB // CHUNK, DIM], fp16, tag="ot")
                for cc_ in range(SLAB // CHUNK):
                    j = s * (SLAB // CHUNK) + cc_
                    gm = psgp.tile([128, DIM], f32, tag="gm")
                    nc.tensor.matmul(
                        gm[:], lhsT=m2[:, j * CHUNK : (j + 1) * CHUNK],
                        rhs=wtr[:], start=True, stop=True,
                    )
                    th = sp.tile([128, DIM], f32, tag="th")
                    nc.scalar.activation(
                        th[:], gm[:], AF.Tanh, bias=0.0, scale=s01[:, j : j + 1]
                    )
                    nc.vector.tensor_scalar(
                        ot[:, cc_, :], th[:], CLAMP, None, ALU.mult
                    )
                dst = out[s * SLAB : (s + 1) * SLAB, :].rearrange(
                    "(c p) d -> p c d", p=128
                )
                nc.sync.dma_start(dst, ot[:])

    nc.compile()
    return nc


def _get_nc():
    global _nc_cache
    if _nc_cache is None:
        _nc_cache = _build()
    return _nc_cache


def kernel(v, U_full, W_full, w1, b1, w2, b2):
    global _last_in_maps
    from concourse.bass_utils import run_bass_kernel_spmd

    v = np.ascontiguousarray(v, dtype=np.float32)
    vt_full = np.ascontiguousarray(
        v.reshape(TOKENS, DIM).T.astype(np.float16)
    )  # [512, 32768] fp16

    uw1 = np.ascontiguousarray(
        np.concatenate([U_full, w1], axis=1), dtype=np.float16
    )                                                          # [512, 96]
    wt = np.ascontiguousarray(W_full.T, dtype=np.float32)      # [64, 512]
    w2c = np.ascontiguousarray(w2, dtype=np.float16).reshape(HID, 1)
    b1c = np.ascontiguousarray(b1, dtype=np.float32).reshape(HID, 1)
    b2h = (np.asarray(b2, dtype=np.float32) * 0.5).reshape(1, 1)
    iota = np.arange(MAX_RANK, dtype=np.float32).reshape(MAX_RANK, 1)
    onesrow = np.ones((1, MAX_RANK), dtype=np.float32)
    ones8 = np.ones((NCORES, 2), dtype=np.float32)
    # pfx[r, c] = 1 if r < min(4 + c, 64)  (norm2 prefix masks, eff = 4..64)
    effs = np.minimum(4 + np.arange(NCAND), MAX_RANK)
    pfxm = (np.arange(MAX_RANK)[:, None] < effs[None, :]).astype(np.float32)

    in_maps = []
    for i in range(NCORES):
        in_maps.append({
            "vt": np.ascontiguousarray(vt_full[:, i * T : (i + 1) * T]),
            "uw1": uw1,
            "wt": wt,
            "w2": w2c,
            "b1": b1c,
            "b2h": b2h,
            "iota": iota,
            "onesrow": onesrow,
            "ones8": ones8,
            "pfx": pfxm,
        })

    _last_in_maps = in_maps
    nc = _get_nc()
    res = run_bass_kernel_spmd(nc, in_maps, core_ids=list(range(NCORES)))
    full = np.concatenate([res.results[i]["out"] for i in range(NCORES)], axis=0)
    return full.reshape(BATCH, SEQ, DIM).astype(np.float32)


# revision 11
# speedup vs baseline: 1.0075x; 1.0075x over previous
"""AdaptiveRankChristoffel kernel for one TRN2 chip (8 NeuronCores).

Data-parallel over tokens: v [4,8192,512] -> 32768 tokens, 4096 per core.
Host pre-transposes v to dim-major fp16 [512, tokens] so the device streams
it straight into matmuls (contraction dim on partitions, no on-chip
transpose); output is written fp16 and widened on the host. All heavy
matmuls run in fp16/f32r at full PE rate; everything accumulates in fp32.

Device pipeline per core:
  pass 1  : stream vT slabs; fused [U|w1] fp16 matmul gives proj + h in one
            PSUM tile; ACT Square writes proj^2 (squn, f32r) straight to
            SBUF; relu -> w2 matmul -> tanh(z/2) (sigmoid rewritten via
            tanh; same ACT table as relu) with accum_out partial sums.
            Interleaved: norm2 candidates for ALL 61 possible eff_ranks
            (static prefix-mask rhs, f32r) -- mask-independent, so it hides
            under pass 1 + the collective.
  exchange: per-core sum -> AllToAll gather (cheaper than AllReduce here)
            -> PE ones-matmul sum -> e = 35.2 + S*c.
  post    : rank mask via exact integer-threshold compares; norm2 column
            selected from the 61 candidates with a register-backed dynamic
            slice; scale = 0.1/(1+sqrt(norm2)+eps).
  pass 2  : m2 = squn*mask (f32r); gamma [128,512] = m2^T @ W^T per
            128-token chunk; out = 10*tanh(gamma*scale) via per-partition
            AP scale on ACT; fp16 out stream.
"""

import sys

sys.path.insert(0, "/opt/trn_rl_repo")

import numpy as np

BATCH, SEQ, DIM = 4, 8192, 512
MAX_RANK = 64
HID = 32
NCORES = 8
TOKENS = BATCH * SEQ            # 32768
T = TOKENS // NCORES            # 4096 tokens per core
SLAB = 512                      # tokens per slab
NSLAB = T // SLAB               # 8
CHUNK = 128                     # tokens per gamma matmul
NCHUNK = T // CHUNK             # 32
KC = DIM // 128                 # 4 contraction chunks
NCAND = 64                      # norm2 candidate columns (eff = 4+min(c,60))

EPS = 1e-8
CLAMP = 10.0
# e = 64*avg_ratio = 35.2 + S * (57.6/65536), S = global sum of tanh(z/2)
E_SCALE = 57.6 / 65536.0
E_BIAS = 35.2

_nc_cache = None
_last_in_maps = None


def _build():
    from concourse import bacc, bass, mybir, tile

    f32 = mybir.dt.float32
    f32r = mybir.dt.float32r
    fp16 = mybir.dt.float16
    i32 = mybir.dt.int32
    AF = mybir.ActivationFunctionType
    ALU = mybir.AluOpType

    nc = bacc.Bacc(None, debug=False)

    vt = nc.declare_dram_parameter("vt", [NSLAB * DIM, SLAB], fp16, isOutput=False)
    uw1 = nc.declare_dram_parameter("uw1", [DIM, MAX_RANK + HID], fp16, isOutput=False)
    wt = nc.declare_dram_parameter("wt", [MAX_RANK, DIM], f32, isOutput=False)
    w2 = nc.declare_dram_parameter("w2", [HID, 1], fp16, isOutput=False)
    b1 = nc.declare_dram_parameter("b1", [HID, 1], f32, isOutput=False)
    b2h = nc.declare_dram_parameter("b2h", [1, 1], f32, isOutput=False)
    iota = nc.declare_dram_parameter("iota", [MAX_RANK, 1], f32, isOutput=False)
    onesrow = nc.declare_dram_parameter("onesrow", [1, MAX_RANK], f32, isOutput=False)
    ones8 = nc.declare_dram_parameter("ones8", [NCORES, 2], f32, isOutput=False)
    pfx = nc.declare_dram_parameter("pfx", [MAX_RANK, NCAND], f32, isOutput=False)
    out = nc.declare_dram_parameter("out", [T, DIM], fp16, isOutput=True)

    with tile.TileContext(nc) as tc:
        with (
            tc.tile_pool(name="persist", bufs=1) as pp,
            tc.tile_pool(name="vtp", bufs=4) as vtp,
            tc.tile_pool(name="small", bufs=2) as sp,
            tc.tile_pool(name="outp", bufs=2) as op_,
            tc.tile_pool(name="ps1", bufs=3, space="PSUM") as ps1p,
            tc.tile_pool(name="ps2", bufs=2, space="PSUM") as ps2p,
            tc.tile_pool(name="psg", bufs=3, space="PSUM") as psgp,
            tc.tile_pool(name="dram", bufs=1, space="DRAM") as dram,
        ):
            # ---- constants ----
            uw1t = pp.tile([128, KC, MAX_RANK + HID], fp16, tag="uw1t")
            nc.gpsimd.dma_start(uw1t[:], uw1[:].rearrange("(c p) m -> p c m", p=128))
            wtr = pp.tile([MAX_RANK, DIM], f32r, tag="wtr")
            nc.gpsimd.dma_start(wtr[:], wt[:])
            w2t = pp.tile([HID, 1], fp16, tag="w2t")
            nc.gpsimd.dma_start(w2t[:], w2[:])
            b1t = pp.tile([HID, 1], f32, tag="b1t")
            nc.gpsimd.dma_start(b1t[:], b1[:])
            b2t = pp.tile([1, 1], f32, tag="b2t")
            nc.gpsimd.dma_start(b2t[:], b2h[:])
            iot = pp.tile([MAX_RANK, 1], f32, tag="iot")
            nc.gpsimd.dma_start(iot[:], iota[:])
            onr = pp.tile([1, MAX_RANK], f32, tag="onr")
            nc.gpsimd.dma_start(onr[:], onesrow[:])
            on8 = pp.tile([NCORES, 2], f32, tag="on8")
            nc.gpsimd.dma_start(on8[:], ones8[:])
            pfxt = pp.tile([MAX_RANK, NCAND], f32r, tag="pfxt")
            nc.gpsimd.dma_start(pfxt[:], pfx[:])

            # ---- persistent state ----
            squn = pp.tile([MAX_RANK, T], f32r, tag="squn")
            m2 = pp.tile([MAX_RANK, T], f32r, tag="m2")
            n2all = pp.tile([128, NCHUNK * NCAND], f32, tag="n2all")
            partials = pp.tile([1, NSLAB], f32, tag="partials")

            # ---- pass 1 (+ interleaved norm2 candidates) ----
            for s in range(NSLAB):
                t0 = s * SLAB
                vslab = vtp.tile([128, KC, SLAB], fp16, tag="vslab")
                src = vt[s * DIM : (s + 1) * DIM, :].rearrange(
                    "(c p) t -> p c t", p=128
                )
                nc.sync.dma_start(vslab[:], src)

                ps1 = ps1p.tile([MAX_RANK + HID, SLAB], f32, tag="ps1")
                for c in range(KC):
                    nc.tensor.matmul(
                        ps1[:], lhsT=uw1t[:, c, :], rhs=vslab[:, c, :],
                        start=(c == 0), stop=(c == KC - 1),
                    )
                # squn = proj^2 straight from PSUM (f32r for downstream matmuls)
                nc.scalar.activation(
                    squn[:, t0 : t0 + SLAB], ps1[0:MAX_RANK, :], AF.Square,
                    bias=0.0, scale=1.0,
                )
                # complexity net
                hrel = sp.tile([HID, SLAB], fp16, tag="hrel")
                nc.vector.tensor_scalar(
                    hrel[:], ps1[MAX_RANK : MAX_RANK + HID, :],
                    b1t[:], 0.0, ALU.add, ALU.max,
                )
                ps2 = ps2p.tile([1, SLAB], f32, tag="ps2share")
                nc.tensor.matmul(ps2[:], lhsT=w2t[:], rhs=hrel[:], start=True, stop=True)
                tval = sp.tile([1, SLAB], f32, tag="tval")
                nc.scalar.activation(
                    tval[:], ps2[:], AF.Tanh, bias=b2t[:], scale=0.5,
                    accum_out=partials[0:1, s : s + 1],
                )
            # ---- local sum -> AllToAll gather -> global sum ----
            gloc0 = pp.tile([1, 1], f32, tag="gloc0")
            nc.vector.reduce_sum(gloc0[:], partials[:], axis=mybir.AxisListType.X)
            glp = ps2p.tile([NCORES, 1], f32, tag="ps2share")
            nc.tensor.matmul(glp[:], lhsT=onr[0:1, 0:NCORES], rhs=gloc0[:],
                             start=True, stop=True)
            gloc = pp.tile([NCORES, 1], f32, tag="gloc")
            nc.vector.tensor_copy(gloc[:], glp[:])
            cci = dram.tile([NCORES, 1], f32)
            cco = dram.tile([NCORES, 1], f32)
            nc.gpsimd.dma_start(cci[:], gloc[:])
            nc.gpsimd.collective_compute(
                "AllToAll", ALU.bypass,
                replica_groups=[list(range(NCORES))],
                ins=[cci[:].opt()], outs=[cco[:].opt()],
            )
            gat = pp.tile([NCORES, 1], f32, tag="gat")
            nc.gpsimd.dma_start(gat[:], cco[:])
            gsp = ps2p.tile([1, 2], f32, tag="ps2share")
            nc.tensor.matmul(gsp[:], lhsT=gat[:], rhs=on8[:], start=True, stop=True)
            gsum = pp.tile([1, 1], f32, tag="gsum")
            nc.vector.tensor_copy(gsum[:], gsp[0:1, 0:1])

            # warm the Sqrt ACT table while the collective is in flight
            scratch = pp.tile([1, 1], f32, tag="scratch")
            nc.scalar.activation(scratch[:], gloc0[:], AF.Sqrt, bias=0.0, scale=0.0)

            # norm2 candidates for every chunk x all 61 eff_ranks -- mask-
            # independent, so this PE work hides under the collective stall
            for j in range(NCHUNK):
                n2p = ps1p.tile([128, NCAND], f32, tag="ps1")
                nc.tensor.matmul(
                    n2p[:], lhsT=squn[:, j * CHUNK : (j + 1) * CHUNK],
                    rhs=pfxt[:], start=True, stop=True,
                )
                nc.vector.tensor_copy(
                    n2all[:, j * NCAND : (j + 1) * NCAND], n2p[:]
                )

            # ---- e scalar -> rank mask + norm2 column select ----
            e_t = pp.tile([1, 1], f32, tag="e_t")
            nc.vector.tensor_scalar(e_t[:], gsum[:], E_SCALE, E_BIAS, ALU.mult, ALU.add)
            # mask[r] = (e - iota >= 1) | (iota <= 3)
            ebp = ps2p.tile([MAX_RANK, 1], f32, tag="ps2share")
            nc.tensor.matmul(ebp[:], lhsT=onr[:], rhs=e_t[:], start=True, stop=True)
            eb = pp.tile([MAX_RANK, 1], f32, tag="eb")
            nc.vector.tensor_copy(eb[:], ebp[:])
            d_t = pp.tile([MAX_RANK, 1], f32, tag="d_t")
            nc.vector.tensor_sub(d_t[:], eb[:], iot[:])
            ma = pp.tile([MAX_RANK, 1], f32, tag="ma")
            nc.vector.tensor_scalar(ma[:], d_t[:], 1.0, None, ALU.is_ge)
            mb = pp.tile([MAX_RANK, 1], f32, tag="mb")
            nc.vector.tensor_scalar(mb[:], iot[:], 3.0, None, ALU.is_le)
            mask = pp.tile([MAX_RANK, 1], f32, tag="mask")
            nc.vector.tensor_tensor(mask[:], ma[:], mb[:], ALU.max)

            # idx = clip(int(e) - 4, 0, 60) -> register -> dynamic column pick
            idxf = pp.tile([1, 1], f32, tag="idxf")
            nc.vector.tensor_scalar(idxf[:], e_t[:], -4.0, 0.0, ALU.add, ALU.max)
            idxf2 = pp.tile([1, 1], f32, tag="idxf2")
            nc.vector.tensor_scalar(idxf2[:], idxf[:], 60.0, None, ALU.min)
            idxi = pp.tile([1, 1], i32, tag="idxi")
            nc.vector.tensor_copy(idxi[:], idxf2[:])
            regs = nc.alloc_registers()
            nc.regs_load(regs, idxi[0:1, 0:1])
            sv = nc.snap(regs, donate=True, min_val=0, max_val=NCAND - 1)
            n2 = pp.tile([128, NCHUNK], f32, tag="n2")
            n2view = n2all[:].rearrange("p (j k) -> p j k", k=NCAND)
            nc.vector.tensor_copy(n2[:], n2view[:, :, bass.ds(sv, 1)])

            # scale = 0.1 / (1 + sqrt(n2) + eps), per token chunk column
            nrm = pp.tile([128, NCHUNK], f32, tag="nrm")
            nc.scalar.activation(nrm[:], n2[:], AF.Sqrt, bias=0.0, scale=1.0)
            np1 = pp.tile([128, NCHUNK], f32, tag="np1")
            nc.vector.tensor_scalar(np1[:], nrm[:], 1.0 + EPS, None, ALU.add)
            rcp = pp.tile([128, NCHUNK], f32, tag="rcp")
            nc.vector.reciprocal(rcp[:], np1[:])
            s01 = pp.tile([128, NCHUNK], f32, tag="s01")
            nc.vector.tensor_scalar(s01[:], rcp[:], 1.0 / CLAMP, None, ALU.mult)

            # m2 = squn * mask  (per slab)
            for s in range(NSLAB):
                t0 = s * SLAB
                nc.vector.tensor_scalar(
                    m2[:, t0 : t0 + SLAB], squn[:, t0 : t0 + SLAB],
                    mask[:], None, ALU.mult,
                )

            # ---- pass 2: gamma + tanh + scale-out (fp16) ----
            for s in range(NSLAB):
                ot = op_.tile([128, SLAB // CHUNK, DIM], fp16, tag="ot")
                for cc_ in range(SLAB // CHUNK):
                    j = s * (SLAB // CHUNK) + cc_
                    gm = psgp.tile([128, DIM], f32, tag="gm")
                    nc.tensor.matmul(
                        gm[:], lhsT=m2[:, j * CHUNK : (j + 1) * CHUNK],
                        rhs=wtr[:], start=True, stop=True,
                    )
                    th = sp.tile([128, DIM], f32, tag="th")
                    nc.scalar.activation(
                        th[:], gm[:], AF.Tanh, bias=0.0, scale=s01[:, j : j + 1]
                    )
                    nc.vector.tensor_scalar(
                        ot[:, cc_, :], th[:], CLAMP, None, ALU.mult
                    )
                dst = out[s * SLAB : (s + 1) * SLAB, :].rearrange(
                    "(c p) d -> p c d", p=128
                )
                nc.sync.dma_start(dst, ot[:])

    nc.compile()
    return nc


def _get_nc():
    global _nc_cache
    if _nc_cache is None:
        _nc_cache = _build()
    return _nc_cache


def kernel(v, U_full, W_full, w1, b1, w2, b2):
    global _last_in_maps
    from concourse.bass_utils import run_bass_kernel_spmd

    v = np.ascontiguousarray(v, dtype=np.float32)
    vt_full = v.reshape(TOKENS, DIM).T.astype(np.float16)  # [512, 32768]
    # per-core slab-major blocks: [NSLAB*DIM, SLAB], each slab contiguous
    vts = vt_full.reshape(DIM, NCORES, NSLAB, SLAB)

    uw1 = np.ascontiguousarray(
        np.concatenate([U_full, w1], axis=1), dtype=np.float16
    )                                                          # [512, 96]
    wt = np.ascontiguousarray(W_full.T, dtype=np.float32)      # [64, 512]
    w2c = np.ascontiguousarray(w2, dtype=np.float16).reshape(HID, 1)
    b1c = np.ascontiguousarray(b1, dtype=np.float32).reshape(HID, 1)
    b2h = (np.asarray(b2, dtype=np.float32) * 0.5).reshape(1, 1)
    iota = np.arange(MAX_RANK, dtype=np.float32).reshape(MAX_RANK, 1)
    onesrow = np.ones((1, MAX_RANK), dtype=np.float32)
    ones8 = np.ones((NCORES, 2), dtype=np.float32)
    # pfx[r, c] = 1 if r < min(4 + c, 64)  (norm2 prefix masks, eff = 4..64)
    effs = np.minimum(4 + np.arange(NCAND), MAX_RANK)
    pfxm = (np.arange(MAX_RANK)[:, None] < effs[None, :]).astype(np.float32)

    in_maps = []
    for i in range(NCORES):
        in_maps.append({
            "vt": np.ascontiguousarray(
                vts[:, i].transpose(1, 0, 2)).reshape(NSLAB * DIM, SLAB),
            "uw1": uw1,
            "wt": wt,
            "w2": w2c,
            "b1": b1c,
            "b2h": b2h,
            "iota": iota,
            "onesrow": onesrow,
            "ones8": ones8,
            "pfx": pfxm,
        })

    _last_in_maps = in_maps
    nc = _get_nc()
    res = run_bass_kernel_spmd(nc, in_maps, core_ids=list(range(NCORES)))
    full = np.concatenate([res.results[i]["out"] for i in range(NCORES)], axis=0)
    return full.reshape(BATCH, SEQ, DIM).astype(np.float32)


# revision 12
# speedup vs baseline: 1.1314x; 1.1230x over previous
"""AdaptiveRankChristoffel kernel for one TRN2 chip (8 NeuronCores).

Data-parallel over tokens: v [4,8192,512] -> 32768 tokens, 4096 per core.
Host pre-transposes v to dim-major fp16 [512, tokens] so the device streams
it straight into matmuls (contraction dim on partitions, no on-chip
transpose); output is written fp16 and widened on the host. All heavy
matmuls run in fp16/f32r at full PE rate; everything accumulates in fp32.

Device pipeline per core:
  pass 1  : stream vT slabs; fused [U|w1] fp16 matmul gives proj + h in one
            PSUM tile; ACT Square writes proj^2 (squn, f32r) straight to
            SBUF; relu -> w2 matmul -> tanh(z/2) (sigmoid rewritten via
            tanh; same ACT table as relu) with accum_out partial sums.
            Interleaved: norm2 candidates for ALL 61 possible eff_ranks
            (static prefix-mask rhs, f32r) -- mask-independent, so it hides
            under pass 1 + the collective.
  exchange: per-core sum -> AllToAll gather (cheaper than AllReduce here)
            -> PE ones-matmul sum -> e = 35.2 + S*c.
  post    : rank mask via exact integer-threshold compares; norm2 column
            selected from the 61 candidates with a register-backed dynamic
            slice; scale = 0.1/(1+sqrt(norm2)+eps).
  pass 2  : m2 = squn*mask (f32r); gamma [128,512] = m2^T @ W^T per
            128-token chunk; out = 10*tanh(gamma*scale) via per-partition
            AP scale on ACT; fp16 out stream.
"""

import sys

sys.path.insert(0, "/opt/trn_rl_repo")

import numpy as np

BATCH, SEQ, DIM = 4, 8192, 512
MAX_RANK = 64
HID = 32
NCORES = 8
TOKENS = BATCH * SEQ            # 32768
T = TOKENS // NCORES            # 4096 tokens per core
SLAB = 512                      # tokens per slab
NSLAB = T // SLAB               # 8
CHUNK = 128                     # tokens per gamma matmul
NCHUNK = T // CHUNK             # 32
KC = DIM // 128                 # 4 contraction chunks
NCAND = 64                      # norm2 candidate columns (eff = 4+min(c,60))

EPS = 1e-8
CLAMP = 10.0
# e = 64*avg_ratio = 35.2 + S * (57.6/65536), S = global sum of tanh(z/2)
E_SCALE = 57.6 / 65536.0
E_BIAS = 35.2

_nc_cache = None
_last_in_maps = None


def _build():
    from concourse import bacc, bass, mybir, tile

    f32 = mybir.dt.float32
    f32r = mybir.dt.float32r
    fp16 = mybir.dt.float16
    i32 = mybir.dt.int32
    AF = mybir.ActivationFunctionType
    ALU = mybir.AluOpType

    nc = bacc.Bacc(None, debug=False)

    vt = nc.declare_dram_parameter("vt", [NSLAB * DIM, SLAB], fp16, isOutput=False)
    uw1 = nc.declare_dram_parameter("uw1", [DIM, MAX_RANK + HID], fp16, isOutput=False)
    wt = nc.declare_dram_parameter("wt", [MAX_RANK, DIM], f32, isOutput=False)
    w2 = nc.declare_dram_parameter("w2", [HID, 1], fp16, isOutput=False)
    b1 = nc.declare_dram_parameter("b1", [HID, 1], f32, isOutput=False)
    b2h = nc.declare_dram_parameter("b2h", [1, 1], f32, isOutput=False)
    iota = nc.declare_dram_parameter("iota", [MAX_RANK, 1], f32, isOutput=False)
    onesrow = nc.declare_dram_parameter("onesrow", [1, MAX_RANK], f32, isOutput=False)
    ones8 = nc.declare_dram_parameter("ones8", [NCORES, 2], f32, isOutput=False)
    ones64 = nc.declare_dram_parameter("ones64", [MAX_RANK, 2], f32, isOutput=False)
    pfx = nc.declare_dram_parameter("pfx", [MAX_RANK, NCAND], f32, isOutput=False)
    out = nc.declare_dram_parameter("out", [T, DIM], fp16, isOutput=True)

    with tile.TileContext(nc) as tc:
        with (
            tc.tile_pool(name="persist", bufs=1) as pp,
            tc.tile_pool(name="vtp", bufs=4) as vtp,
            tc.tile_pool(name="small", bufs=2) as sp,
            tc.tile_pool(name="outp", bufs=2) as op_,
            tc.tile_pool(name="ps1", bufs=3, space="PSUM") as ps1p,
            tc.tile_pool(name="ps2", bufs=2, space="PSUM") as ps2p,
            tc.tile_pool(name="psg", bufs=3, space="PSUM") as psgp,
            tc.tile_pool(name="dram", bufs=1, space="DRAM") as dram,
        ):
            # ---- constants ----
            uw1t = pp.tile([128, KC, MAX_RANK + HID], fp16, tag="uw1t")
            nc.gpsimd.dma_start(uw1t[:], uw1[:].rearrange("(c p) m -> p c m", p=128))
            wtr = pp.tile([MAX_RANK, DIM], f32r, tag="wtr")
            nc.gpsimd.dma_start(wtr[:], wt[:])
            w2t = pp.tile([HID, 1], fp16, tag="w2t")
            nc.gpsimd.dma_start(w2t[:], w2[:])
            b1t = pp.tile([HID, 1], f32, tag="b1t")
            nc.gpsimd.dma_start(b1t[:], b1[:])
            b2t = pp.tile([1, 1], f32, tag="b2t")
            nc.gpsimd.dma_start(b2t[:], b2h[:])
            iot = pp.tile([MAX_RANK, 1], f32, tag="iot")
            nc.gpsimd.dma_start(iot[:], iota[:])
            onr = pp.tile([1, MAX_RANK], f32, tag="onr")
            nc.gpsimd.dma_start(onr[:], onesrow[:])
            on8 = pp.tile([NCORES, 2], f32, tag="on8")
            nc.gpsimd.dma_start(on8[:], ones8[:])
            o64 = pp.tile([MAX_RANK, 2], f32, tag="o64")
            nc.gpsimd.dma_start(o64[:], ones64[:])
            pfxt = pp.tile([MAX_RANK, NCAND], f32r, tag="pfxt")
            nc.gpsimd.dma_start(pfxt[:], pfx[:])

            # ---- persistent state ----
            squn = pp.tile([MAX_RANK, T], f32r, tag="squn")
            m2 = pp.tile([MAX_RANK, T], f32r, tag="m2")
            n2all = pp.tile([128, NCHUNK * NCAND], f32, tag="n2all")
            partials = pp.tile([1, NSLAB], f32, tag="partials")

            # ---- pass 1 (+ interleaved norm2 candidates) ----
            for s in range(NSLAB):
                t0 = s * SLAB
                vslab = vtp.tile([128, KC, SLAB], fp16, tag="vslab")
                src = vt[s * DIM : (s + 1) * DIM, :].rearrange(
                    "(c p) t -> p c t", p=128
                )
                nc.sync.dma_start(vslab[:], src)

                ps1 = ps1p.tile([MAX_RANK + HID, SLAB], f32, tag="ps1")
                for c in range(KC):
                    nc.tensor.matmul(
                        ps1[:], lhsT=uw1t[:, c, :], rhs=vslab[:, c, :],
                        start=(c == 0), stop=(c == KC - 1),
                    )
                # squn = proj^2 straight from PSUM (f32r for downstream matmuls)
                nc.scalar.activation(
                    squn[:, t0 : t0 + SLAB], ps1[0:MAX_RANK, :], AF.Square,
                    bias=0.0, scale=1.0,
                )
                # complexity net
                hrel = sp.tile([HID, SLAB], fp16, tag="hrel")
                nc.vector.tensor_scalar(
                    hrel[:], ps1[MAX_RANK : MAX_RANK + HID, :],
                    b1t[:], 0.0, ALU.add, ALU.max,
                )
                ps2 = ps2p.tile([1, SLAB], f32, tag="ps2share")
                nc.tensor.matmul(ps2[:], lhsT=w2t[:], rhs=hrel[:], start=True, stop=True)
                tval = sp.tile([1, SLAB], f32, tag="tval")
                nc.scalar.activation(
                    tval[:], ps2[:], AF.Tanh, bias=b2t[:], scale=0.5,
                    accum_out=partials[0:1, s : s + 1],
                )
            # ---- local sum -> AllToAll gather -> global sum ----
            gloc0 = pp.tile([1, 1], f32, tag="gloc0")
            nc.vector.reduce_sum(gloc0[:], partials[:], axis=mybir.AxisListType.X)
            glp = ps2p.tile([NCORES, 1], f32, tag="ps2share")
            nc.tensor.matmul(glp[:], lhsT=onr[0:1, 0:NCORES], rhs=gloc0[:],
                             start=True, stop=True)
            gloc = pp.tile([NCORES, 1], f32, tag="gloc")
            nc.vector.tensor_copy(gloc[:], glp[:])
            cci = dram.tile([NCORES, 1], f32)
            cco = dram.tile([NCORES, 1], f32)
            nc.gpsimd.dma_start(cci[:], gloc[:])
            nc.gpsimd.collective_compute(
                "AllToAll", ALU.bypass,
                replica_groups=[list(range(NCORES))],
                ins=[cci[:].opt()], outs=[cco[:].opt()],
            )
            gat = pp.tile([NCORES, 1], f32, tag="gat")
            nc.gpsimd.dma_start(gat[:], cco[:])
            gsp = ps2p.tile([1, 2], f32, tag="ps2share")
            nc.tensor.matmul(gsp[:], lhsT=gat[:], rhs=on8[:], start=True, stop=True)
            gsum = pp.tile([1, 1], f32, tag="gsum")
            nc.vector.tensor_copy(gsum[:], gsp[0:1, 0:1])

            # norm2 candidates for every chunk x all 61 eff_ranks -- mask-
            # independent, so this PE work hides under the collective stall
            for j in range(NCHUNK):
                n2p = ps1p.tile([128, NCAND], f32, tag="ps1")
                nc.tensor.matmul(
                    n2p[:], lhsT=squn[:, j * CHUNK : (j + 1) * CHUNK],
                    rhs=pfxt[:], start=True, stop=True,
                )
                nc.vector.tensor_copy(
                    n2all[:, j * NCAND : (j + 1) * NCAND], n2p[:]
                )

            n2view = n2all[:].rearrange("p (j k) -> p j k", k=NCAND)

            def emit_mask_scale(e_src, tg):
                """e scalar -> (rank mask, per-chunk scale s01, count k as f32).

                The eff_rank count k is derived from the mask itself (PE
                ones-matmul over partitions), so the dynamic norm2-column
                pick (column k-4) is exactly consistent with the mask."""
                ebp = ps2p.tile([MAX_RANK, 1], f32, tag="ps2share")
                nc.tensor.matmul(ebp[:], lhsT=onr[:], rhs=e_src[:],
                                 start=True, stop=True)
                eb = pp.tile([MAX_RANK, 1], f32, tag=f"eb{tg}")
                nc.vector.tensor_copy(eb[:], ebp[:])
                d_t = pp.tile([MAX_RANK, 1], f32, tag=f"d_t{tg}")
                nc.vector.tensor_sub(d_t[:], eb[:], iot[:])
                ma = pp.tile([MAX_RANK, 1], f32, tag=f"ma{tg}")
                nc.vector.tensor_scalar(ma[:], d_t[:], 1.0, None, ALU.is_ge)
                mb = pp.tile([MAX_RANK, 1], f32, tag=f"mb{tg}")
                nc.vector.tensor_scalar(mb[:], iot[:], 3.0, None, ALU.is_le)
                mask = pp.tile([MAX_RANK, 1], f32, tag=f"mask{tg}")
                nc.vector.tensor_tensor(mask[:], ma[:], mb[:], ALU.max)
                kp = ps2p.tile([1, 2], f32, tag="ps2share")
                nc.tensor.matmul(kp[:], lhsT=mask[:], rhs=o64[:],
                                 start=True, stop=True)
                kf = pp.tile([1, 1], f32, tag=f"kf{tg}")
                nc.vector.tensor_copy(kf[:], kp[0:1, 0:1])
                idxf = pp.tile([1, 1], f32, tag=f"idxf{tg}")
                nc.vector.tensor_scalar(idxf[:], kf[:], -4.0, None, ALU.add)
                idxi = pp.tile([1, 1], i32, tag=f"idxi{tg}")
                nc.vector.tensor_copy(idxi[:], idxf[:])
                regs = nc.alloc_registers()
                nc.regs_load(regs, idxi[0:1, 0:1])
                sv = nc.snap(regs, donate=True, min_val=0, max_val=NCAND - 1)
                n2 = pp.tile([128, NCHUNK], f32, tag=f"n2{tg}")
                nc.vector.tensor_copy(n2[:], n2view[:, :, bass.ds(sv, 1)])
                nrm = pp.tile([128, NCHUNK], f32, tag=f"nrm{tg}")
                nc.scalar.activation(nrm[:], n2[:], AF.Sqrt, bias=0.0, scale=1.0)
                np1 = pp.tile([128, NCHUNK], f32, tag=f"np1{tg}")
                nc.vector.tensor_scalar(np1[:], nrm[:], 1.0 + EPS, None, ALU.add)
                rcp = pp.tile([128, NCHUNK], f32, tag=f"rcp{tg}")
                nc.vector.reciprocal(rcp[:], np1[:])
                s01 = pp.tile([128, NCHUNK], f32, tag=f"s01{tg}")
                nc.vector.tensor_scalar(s01[:], rcp[:], 1.0 / CLAMP, None, ALU.mult)
                return mask, s01, kf

            def emit_pass2(mask, s01):
                """m2 = squn*mask, gamma, out = 10*tanh(gamma*scale), stream."""
                for s in range(NSLAB):
                    t0 = s * SLAB
                    nc.vector.tensor_scalar(
                        m2[:, t0 : t0 + SLAB], squn[:, t0 : t0 + SLAB],
                        mask[:], None, ALU.mult,
                    )
                for s in range(NSLAB):
                    ot = op_.tile([128, SLAB // CHUNK, DIM], fp16, tag="ot")
                    for cc_ in range(SLAB // CHUNK):
                        j = s * (SLAB // CHUNK) + cc_
                        gm = psgp.tile([128, DIM], f32, tag="gm")
                        nc.tensor.matmul(
                            gm[:], lhsT=m2[:, j * CHUNK : (j + 1) * CHUNK],
                            rhs=wtr[:], start=True, stop=True,
                        )
                        th = sp.tile([128, DIM], f32, tag="th")
                        nc.scalar.activation(
                            th[:], gm[:], AF.Tanh, bias=0.0,
                            scale=s01[:, j : j + 1],
                        )
                        nc.vector.tensor_scalar(
                            ot[:, cc_, :], th[:], CLAMP, None, ALU.mult
                        )
                    dst = out[s * SLAB : (s + 1) * SLAB, :].rearrange(
                        "(c p) d -> p c d", p=128
                    )
                    nc.sync.dma_start(dst, ot[:])

            # ---- speculative pass 2 with the local-shard eff estimate ----
            # (hides entirely under the collective; exact for any input via
            # the verified redo below)
            el = pp.tile([1, 1], f32, tag="el")
            nc.vector.tensor_scalar(el[:], gloc0[:], NCORES * E_SCALE, E_BIAS,
                                    ALU.mult, ALU.add)
            mask_l, s01_l, kf_l = emit_mask_scale(el, "l")
            emit_pass2(mask_l, s01_l)

            # ---- verify against the global sum; redo exactly on mismatch ----
            eg = pp.tile([1, 1], f32, tag="eg")
            nc.vector.tensor_scalar(eg[:], gsum[:], E_SCALE, E_BIAS,
                                    ALU.mult, ALU.add)
            mask_g, s01_g, kf_g = emit_mask_scale(eg, "g")
            eqf = pp.tile([1, 1], f32, tag="eqf")
            nc.vector.tensor_tensor(eqf[:], kf_l[:], kf_g[:], ALU.is_equal)
            eqi = pp.tile([1, 1], i32, tag="eqi")
            nc.vector.tensor_copy(eqi[:], eqf[:])
            cregs = nc.alloc_registers()
            nc.regs_load(cregs, eqi[0:1, 0:1])
            csv = nc.snap(cregs, donate=True, min_val=0, max_val=1)
            with tc.If(csv == 0):
                emit_pass2(mask_g, s01_g)

    nc.compile()
    return nc


def _get_nc():
    global _nc_cache
    if _nc_cache is None:
        _nc_cache = _build()
    return _nc_cache


def kernel(v, U_full, W_full, w1, b1, w2, b2):
    global _last_in_maps
    from concourse.bass_utils import run_bass_kernel_spmd

    v = np.ascontiguousarray(v, dtype=np.float32)
    vt_full = v.reshape(TOKENS, DIM).T.astype(np.float16)  # [512, 32768]
    # per-core slab-major blocks: [NSLAB*DIM, SLAB], each slab contiguous
    vts = vt_full.reshape(DIM, NCORES, NSLAB, SLAB)

    uw1 = np.ascontiguousarray(
        np.concatenate([U_full, w1], axis=1), dtype=np.float16
    )                                                          # [512, 96]
    wt = np.ascontiguousarray(W_full.T, dtype=np.float32)      # [64, 512]
    w2c = np.ascontiguousarray(w2, dtype=np.float16).reshape(HID, 1)
    b1c = np.ascontiguousarray(b1, dtype=np.float32).reshape(HID, 1)
    b2h = (np.asarray(b2, dtype=np.float32) * 0.5).reshape(1, 1)
    iota = np.arange(MAX_RANK, dtype=np.float32).reshape(MAX_RANK, 1)
    onesrow = np.ones((1, MAX_RANK), dtype=np.float32)
    ones8 = np.ones((NCORES, 2), dtype=np.float32)
    # pfx[r, c] = 1 if r < min(4 + c, 64)  (norm2 prefix masks, eff = 4..64)
    effs = np.minimum(4 + np.arange(NCAND), MAX_RANK)
    pfxm = (np.arange(MAX_RANK)[:, None] < effs[None, :]).astype(np.float32)

    in_maps = []
    for i in range(NCORES):
        in_maps.append({
            "vt": np.ascontiguousarray(
                vts[:, i].transpose(1, 0, 2)).reshape(NSLAB * DIM, SLAB),
            "uw1": uw1,
            "wt": wt,
            "w2": w2c,
            "b1": b1c,
            "b2h": b2h,
            "iota": iota,
            "onesrow": onesrow,
            "ones8": ones8,
            "ones64": np.ones((MAX_RANK, 2), dtype=np.float32),
            "pfx": pfxm,
        })

    _last_in_maps = in_maps
    nc = _get_nc()
    res = run_bass_kernel_spmd(nc, in_maps, core_ids=list(range(NCORES)))
    full = np.concatenate([res.results[i]["out"] for i in range(NCORES)], axis=0)
    return full.reshape(BATCH, SEQ, DIM).astype(np.float32)
